# revision 1
# baseline (speedup 1.0000x reference)
"""Self-contained Trainium2 Bass kernel for nn_ContrastiveModule.

Reference computation (b=2, in_ch=256, h=w=64, c=32):
  branch(x, W, g, b) = relu(instancenorm(W @ x) * g + b)   (1x1 conv + IN + relu)
  Q_t = branch(x_t, W1), K_t = branch(x_t, W2), V_t = branch(x_t, W3)
  A_uv = softmax(Q_u^T K_v) per sample (softmax over last axis)
  outputs: chained weighted column-sums of the A matrices (p1a, p1b, p2,
  p3a, p3b broadcast over channels) plus v1, v2, v3.

Kernel strategy (no collectives; tiny host-side reductions between launches):
  A pass over matrix A computes out_m[j] = sum_i v_m[i]/l_i * exp(S[i,j]+B)
  streaming 128-row blocks of S = Q^T K: matmul -> exp on ScalarE (constant
  bias B; exact softmax since B cancels; S >= 0 and bounded so no row-max
  needed) -> row-sums via activation accum_out -> weighted column-sums by
  using E-chunks as matmul *weights* against C=[v0/l, v1/l], accumulated in
  a single PSUM bank.

  L1 (NEFF-A, 8 cores): A12/A13 colsum passes (2-way row split) + all
      branch products (Q/K/V) as outputs.
  L2..L4 (NEFF-B): A23, A32, A21 passes (4-way row split per sample).
"""

import os
import numpy as np
import concourse.bacc as bacc
import concourse.tile as tile
from concourse import mybir
from concourse.bass_utils import run_bass_kernel_spmd

F32 = mybir.dt.float32
BF16 = mybir.dt.bfloat16
HW = 4096          # h*w
C = 32             # branch out-channels
INCH = 256         # in channels
EXP_BIAS = -40.0   # exp(S + bias): real S in [0, ~37]; bias cancels in softmax
ALU = mybir.AluOpType
ACTF = mybir.ActivationFunctionType

# Matmul dtype: float32r streams 1 col/cycle (vs 4 for float32) with
# ~1e-4 relative error; bf16 E dominates the error budget (~1e-3).
S_DTYPE = mybir.dt.float32r

L1_R = 2048        # rows per core in launch 1
LN_R = 1024        # rows per core in launches 2-4

# Per-launch HW exec time (ns) when tracing is enabled via kernel(trace=True)
LAST_EXEC_NS = []


def _np_for(dt):
    return mybir.dt.np(dt)


def _mm(nc, out_ps, lhsT, rhs, **kw):
    nc.tensor.matmul(out_ps, lhsT=lhsT.bitcast(S_DTYPE), rhs=rhs.bitcast(S_DTYPE), **kw)


# --------------------------------------------------------------------------
# Device-side building blocks
# --------------------------------------------------------------------------

def _branch_matmul(nc, y_ps, wT_sb, x_sb, n):
    """y_ps [C, n] (PSUM) = W @ x.  wT_sb: [128, 2, C] (W^T chunked),
    x_sb: [128, 2, n] (x chunked: channel = t*128 + p)."""
    for kc in range(2):
        for j in range(0, n, 512):
            _mm(nc, y_ps[:, j:j + 512], wT_sb[:, kc, :], x_sb[:, kc, j:j + 512],
                start=(kc == 0), stop=(kc == 1))


def _affine_from_stats(nc, pool, stats, g_sb, b_sb, eps=1e-5):
    """bn_aggr + instance-norm affine fold: returns (s, t), norm(y) = s*y + t."""
    mv = pool.tile([C, nc.vector.BN_AGGR_DIM], F32, tag="bnaggr")
    nc.vector.bn_aggr(out=mv, in_=stats)
    # rstd = rsqrt(var+eps) via DVE-only Newton iteration (seed 1/(var+eps));
    # var ~= 1 for this data, converges well below fp32 eps in 5 iters. Using
    # ACT Sqrt/Ln here would force ~2.7us activation-table switches.
    vx = pool.tile([C, 1], F32, tag="vx")
    nc.vector.tensor_scalar_add(out=vx, in0=mv[:, 1:2], scalar1=eps)
    rstd = pool.tile([C, 1], F32, tag="rstd")
    nc.vector.reciprocal(out=rstd, in_=vx)
    nt = pool.tile([C, 1], F32, tag="nt")
    for _ in range(5):
        nc.vector.tensor_mul(out=nt, in0=rstd, in1=rstd)
        nc.vector.tensor_mul(out=nt, in0=nt, in1=vx)
        nc.vector.tensor_scalar(out=nt, in0=nt, scalar1=-0.5, scalar2=1.5,
                                op0=ALU.mult, op1=ALU.add)
        nc.vector.tensor_mul(out=rstd, in0=rstd, in1=nt)
    s_aff = pool.tile([C, 1], F32, tag="saff")
    nc.vector.tensor_mul(out=s_aff, in0=g_sb, in1=rstd)
    t_aff = pool.tile([C, 1], F32, tag="taff")
    nc.vector.tensor_mul(out=t_aff, in0=mv[:, 0:1], in1=s_aff)
    nc.vector.tensor_sub(out=t_aff, in0=b_sb, in1=t_aff)
    return s_aff, t_aff


def _norm_params(nc, pool, y_ps, g_sb, b_sb, eps=1e-5):
    n = y_ps.shape[-1]
    nchunk = n // 512
    stats = pool.tile([C, nchunk, nc.vector.BN_STATS_DIM], F32, tag="bnstats")
    for i in range(nchunk):
        nc.vector.bn_stats(out=stats[:, i, :], in_=y_ps[:, i * 512:(i + 1) * 512])
    return _affine_from_stats(nc, pool, stats, g_sb, b_sb, eps)


def _softmax_pass(nc, sbuf, psum, q_sb, k_sb, v_sb, acc_ps, nblk, skip=(), interleave=None):
    """q_sb: [C, R] f32 (lhsT layout), k_sb: [C, HW] f32,
    v_sb: [128, nblk, 2] f32 row weights, acc_ps: [128, 32, 2] f32 PSUM.
    Accumulates acc_ps[p, c, m] += sum_i v_m[i]/l_i * exp(S[i, 128c+p]+B)."""
    SPANS = (1536, 1536, 1024)
    nspan = len(SPANS)
    bias_sb = sbuf.tile([128, 1], F32, tag="expbias")
    nc.vector.memset(bias_sb, EXP_BIAS)
    for blk in range(nblk):
        if interleave is not None:
            interleave(blk)
        e_sb = sbuf.tile([128, HW], BF16, tag="E")
        lpart = sbuf.tile([128, nspan], F32, tag="lpart")
        j0 = 0
        for s, span in enumerate(SPANS):
            s_ps = psum.tile([128, 1536], F32, tag="S")
            for c2 in range(span // 512):
                j = j0 + c2 * 512
                _mm(nc, s_ps[:, c2 * 512:(c2 + 1) * 512],
                    q_sb[:, blk * 128:(blk + 1) * 128], k_sb[:, j:j + 512],
                    start=True, stop=True)
            nc.scalar.activation(
                out=e_sb[:, j0:j0 + span], in_=s_ps[:, :span],
                func=ACTF.Exp, bias=bias_sb, scale=1.0,
                accum_out=lpart[:, s:s + 1])
            j0 += span
        l = sbuf.tile([128, 1], F32, tag="l")
        nc.vector.tensor_reduce(out=l, in_=lpart, axis=mybir.AxisListType.X, op=ALU.add)
        rl = sbuf.tile([128, 1], F32, tag="rl")
        nc.vector.reciprocal(out=rl, in_=l)
        cs = sbuf.tile([128, 2], BF16, tag="cs")
        nc.vector.tensor_scalar_mul(out=cs[:, 0:1], in0=v_sb[:, blk, 0:1], scalar1=rl)
        nc.vector.tensor_scalar_mul(out=cs[:, 1:2], in0=v_sb[:, blk, 1:2], scalar1=rl)
        for c in (range(32) if "colsum" not in skip else ()):
            # start=True clears has_written for the WHOLE bank, so only the
            # very first chunk-matmul may use it; block-0 chunks c>0 land on
            # cleared bits and overwrite (then set bits), blocks 1+ accumulate.
            nc.tensor.matmul(
                acc_ps[:, c, :],
                lhsT=e_sb[:, c * 128:(c + 1) * 128],
                rhs=cs,
                start=(blk == 0 and c == 0),
                stop=(blk == nblk - 1 and c == 31),
                skip_group_check=True)


# --------------------------------------------------------------------------
# NEFF-B: pure pass kernel (launches 2-4)
# --------------------------------------------------------------------------

def build_pass_kernel(R=LN_R, repeat=1, hw_loop=False, skip=()):
    nblk = R // 128
    nc = bacc.Bacc("TRN2", num_devices=8)
    q = nc.dram_tensor("q", [C, R], S_DTYPE, kind="ExternalInput")
    k = nc.dram_tensor("k", [C, HW], S_DTYPE, kind="ExternalInput")
    v = nc.dram_tensor("v", [R, 2], F32, kind="ExternalInput")
    out = nc.dram_tensor("out", [32, 128, 2], F32, kind="ExternalOutput")
    with tile.TileContext(nc) as tc:
        with (
            tc.tile_pool(name="sbuf", bufs=2) as sbuf,
            tc.tile_pool(name="sing", bufs=1) as sing,
            tc.tile_pool(name="psum", bufs=2, space="PSUM") as psum,
            tc.tile_pool(name="psacc", bufs=1, space="PSUM") as psacc,
        ):
            warm = sing.tile([1, 1], F32)
            nc.vector.memset(warm, 1.0)
            nc.scalar.activation(out=warm, in_=warm, func=ACTF.Exp, bias=0.0)
            q_sb = sing.tile([C, R], S_DTYPE)
            nc.sync.dma_start(out=q_sb, in_=q[:, :])
            k_sb = sing.tile([C, HW], S_DTYPE)
            for kc in range(0, HW, 1536):
                ke = min(HW, kc + 1536)
                nc.sync.dma_start(out=k_sb[:, kc:ke], in_=k[:, kc:ke])
            v_sb = sing.tile([128, nblk, 2], F32)
            nc.sync.dma_start(out=v_sb, in_=v.rearrange("(n p) m -> p n m", p=128))
            for _rep in range(repeat):
                    acc_ps = psacc.tile([128, 32, 2], F32)
                    _softmax_pass(nc, sbuf, psum, q_sb, k_sb, v_sb, acc_ps, nblk)
                    acc_sb = sing.tile([128, 32, 2], F32)
                    nc.vector.tensor_copy(out=acc_sb, in_=acc_ps)
                    nc.sync.dma_start(out=out.rearrange("c p m -> p c m"), in_=acc_sb)
    nc.compile()
    return nc


# --------------------------------------------------------------------------
# NEFF-A: branch computation + colsum pass (launch 1)
# --------------------------------------------------------------------------

def build_l1_kernel(R=L1_R, repeat=1):
    nblk = R // 128
    nc = bacc.Bacc("TRN2", num_devices=8)
    xq = nc.dram_tensor("xq", [INCH, HW], S_DTYPE, kind="ExternalInput")
    xqr = nc.dram_tensor("xqr", [INCH, R], S_DTYPE, kind="ExternalInput")
    xk = nc.dram_tensor("xk", [INCH, HW], S_DTYPE, kind="ExternalInput")
    xv1 = nc.dram_tensor("xv1", [INCH, HW], S_DTYPE, kind="ExternalInput")
    xv2 = nc.dram_tensor("xv2", [INCH, HW], S_DTYPE, kind="ExternalInput")
    wqT = nc.dram_tensor("wqT", [INCH, C], S_DTYPE, kind="ExternalInput")
    wkT = nc.dram_tensor("wkT", [INCH, C], S_DTYPE, kind="ExternalInput")
    wv1T = nc.dram_tensor("wv1T", [INCH, C], S_DTYPE, kind="ExternalInput")
    wv2T = nc.dram_tensor("wv2T", [INCH, C], S_DTYPE, kind="ExternalInput")
    # affine params, columns: gq, bq, gk, bk, gv1, bv1, gv2, bv2
    prm = nc.dram_tensor("prm", [C, 8], F32, kind="ExternalInput")
    kfull = nc.dram_tensor("kfull", [C, HW], S_DTYPE, kind="ExternalOutput")
    vout1 = nc.dram_tensor("vout1", [C, HW], F32, kind="ExternalOutput")
    vout2 = nc.dram_tensor("vout2", [C, HW], F32, kind="ExternalOutput")
    out = nc.dram_tensor("out", [32, 128, 2], F32, kind="ExternalOutput")

    def _load_x(pool, dram, n, tag, chunks=1):
        t = pool.tile([128, 2, n], S_DTYPE, tag=tag)
        src = dram.rearrange("(t p) n -> p t n", p=128)
        step = n // chunks
        for j in range(0, n, step):
            nc.sync.dma_start(out=t[:, :, j:j + step], in_=src[:, :, j:j + step])
        return t

    def _load_w(pool, dram, tag):
        t = pool.tile([128, 2, C], S_DTYPE, tag=tag)
        nc.sync.dma_start(out=t, in_=dram.rearrange("(t p) m -> p t m", p=128))
        return t

    with tile.TileContext(nc) as tc:
        with (
            tc.tile_pool(name="xbuf", bufs=2) as xbuf,
            tc.tile_pool(name="xcbuf", bufs=6) as xcbuf,
            tc.tile_pool(name="small", bufs=2) as small,
            tc.tile_pool(name="sbuf", bufs=2) as sbuf,
            tc.tile_pool(name="sing", bufs=1) as sing,
        ):
            prm_sb = sing.tile([C, 8], F32)
            nc.sync.dma_start(out=prm_sb, in_=prm[:, :])
            warm = sing.tile([1, 1], F32)
            nc.vector.memset(warm, 1.0)
            nc.scalar.activation(out=warm, in_=warm, func=ACTF.Exp, bias=0.0)
            k_sb = sing.tile([C, HW], S_DTYPE)
            qr_sb = sing.tile([C, R], S_DTYPE)
            v1_sb = sing.tile([C, HW], F32)
            v2_sb = sing.tile([C, HW], F32)

            def _load_x_chunks(dram, n, tag, step=512):
                # separate chunk tiles -> per-chunk DMA deps (stream compute)
                src = dram.rearrange("(t p) n -> p t n", p=128)
                ts = []
                for j in range(0, n, step):
                    t = xcbuf.tile([128, 2, step], S_DTYPE, tag=tag)
                    nc.sync.dma_start(out=t, in_=src[:, :, j:j + step])
                    ts.append(t)
                return ts

            def _branch_stream(y_pool, xch, w_t, g, b, out_sb, n=HW):
                # per 512-col chunk: 2 accumulating matmuls then bn_stats,
                # so stats complete right after the last DMA chunk lands
                y_ps = y_pool.tile([C, n], F32, tag="y")
                stats = small.tile([C, n // 512, nc.vector.BN_STATS_DIM], F32,
                                   tag="bnstats")
                for ci, x_t in enumerate(xch):
                    j = ci * 512
                    for kc in range(2):
                        _mm(nc, y_ps[:, j:j + 512], w_t[:, kc, :],
                            x_t[:, kc, :], start=(kc == 0), stop=(kc == 1))
                    nc.vector.bn_stats(out=stats[:, ci, :], in_=y_ps[:, j:j + 512])
                s_aff, t_aff = _affine_from_stats(nc, small, stats, g, b)
                if out_sb is not None:
                    nc.scalar.activation(out=out_sb, in_=y_ps, func=ACTF.Relu,
                                         bias=t_aff, scale=s_aff)
                return s_aff, t_aff

            for _rep in range(repeat):
              with tc.tile_pool(name="ypsum", bufs=1, space="PSUM") as ypsum:
                # K branch (needed by the pass), Q branch (stats only -> the
                # affine params for the Qr row-slice; Q-full has no consumer)
                wk_t = _load_w(small, wkT, "w")
                wq_t = _load_w(small, wqT, "w")
                xk_ch = _load_x_chunks(xk, HW, "xc")
                _branch_stream(ypsum, xk_ch, wk_t, prm_sb[:, 2:3], prm_sb[:, 3:4], k_sb)
                nc.sync.dma_start(out=kfull[:, :], in_=k_sb)

                xq_ch = _load_x_chunks(xq, HW, "xc")
                s_q, t_q = _branch_stream(ypsum, xq_ch, wq_t,
                                          prm_sb[:, 0:1], prm_sb[:, 1:2], None)

                xqr_ch = _load_x_chunks(xqr, R, "xc")
                yr_ps = ypsum.tile([C, R], F32, tag="y")
                for ci, x_t in enumerate(xqr_ch):
                    j = ci * 512
                    for kc in range(2):
                        _mm(nc, yr_ps[:, j:j + 512], wq_t[:, kc, :],
                            x_t[:, kc, :], start=(kc == 0), stop=(kc == 1))
                nc.scalar.activation(out=qr_sb, in_=yr_ps, func=ACTF.Relu,
                                     bias=t_q, scale=s_q)

            with (
                tc.tile_pool(name="pspass", bufs=2, space="PSUM") as psum,
                tc.tile_pool(name="psacc", bufs=1, space="PSUM") as psacc,
                tc.tile_pool(name="ybrps", bufs=1, space="PSUM") as ybr,
            ):
                v_sb = sing.tile([128, nblk, 2], F32)
                nc.vector.memset(v_sb[:, :, 0:1], 1.0)
                nc.vector.memset(v_sb[:, :, 1:2], 0.0)
                acc_ps = psacc.tile([128, 32, 2], F32)

                # v1/v2 extra branches, interleaved chunkwise with the pass
                # loop below (1-bank PSUM; matmul -> DVE copy; stats + relu
                # emitted at the interleave points after their last chunk).
                vstate = {}

                def _v_setup(slot, xdram, wdram, gcol, out_sb, outdram):
                    x_t = _load_x(xbuf, xdram, HW, "x")
                    w_t = _load_w(small, wdram, "w")
                    y_sb = sing.tile([C, HW], F32, tag=f"ybr_sb{slot}")
                    vstate[slot] = (x_t, w_t, y_sb, gcol, out_sb, outdram)

                def _v_chunk(slot, j):
                    x_t, w_t, y_sb, _, _, _ = vstate[slot]
                    y_ps = ybr.tile([C, 512], F32, tag="ybr")
                    for kc in range(2):
                        _mm(nc, y_ps, w_t[:, kc, :], x_t[:, kc, j:j + 512],
                            start=(kc == 0), stop=(kc == 1))
                    nc.vector.tensor_copy(out=y_sb[:, j:j + 512], in_=y_ps)

                def _v_finish(slot):
                    # affine+relu on DVE -- ACT is saturated by the pass exp
                    _, _, y_sb, gcol, out_sb, outdram = vstate[slot]
                    s_aff, t_aff = _norm_params(
                        nc, small, y_sb, prm_sb[:, gcol:gcol + 1],
                        prm_sb[:, gcol + 1:gcol + 2])
                    nc.vector.tensor_scalar(out=out_sb, in0=y_sb,
                                            scalar1=s_aff, scalar2=t_aff,
                                            op0=ALU.mult, op1=ALU.add)
                    nc.vector.tensor_scalar_max(out=out_sb, in0=out_sb, scalar1=0.0)
                    nc.sync.dma_start(out=outdram[:, :], in_=out_sb)

                def _interleave(blk):
                    # 2 chunks per block: v1 during blks 0-3 (finish at 4),
                    # v2 during blks 4-7 (finish at 8); tail stays clean.
                    if blk == 0:
                        _v_setup(0, xv1, wv1T, 4, v1_sb, vout1)
                    if blk < 4:
                        _v_chunk(0, 1024 * blk)
                        _v_chunk(0, 1024 * blk + 512)
                    if blk == 4:
                        _v_finish(0)
                        _v_setup(1, xv2, wv2T, 6, v2_sb, vout2)
                    if 4 <= blk < 8:
                        _v_chunk(1, 1024 * (blk - 4))
                        _v_chunk(1, 1024 * (blk - 4) + 512)
                    if blk == 8:
                        _v_finish(1)

                _softmax_pass(nc, sbuf, psum, qr_sb, k_sb, v_sb, acc_ps, nblk,
                              interleave=_interleave)
                acc_sb = sing.tile([128, 32, 2], F32)
                nc.vector.tensor_copy(out=acc_sb, in_=acc_ps)
                nc.sync.dma_start(out=out.rearrange("c p m -> p c m"), in_=acc_sb)
    nc.compile()
    return nc


# --------------------------------------------------------------------------
# Host-side orchestration
# --------------------------------------------------------------------------

_cache = {}


def _get_kernels():
    if "l1" not in _cache:
        _cache["l1"] = build_l1_kernel()
    if "pass" not in _cache:
        _cache["pass"] = build_pass_kernel()
    return _cache["l1"], _cache["pass"]


def _run(nc, in_maps, trace):
    res = run_bass_kernel_spmd(nc, in_maps, core_ids=list(range(8)), trace=trace)
    if trace:
        LAST_EXEC_NS.append(res.exec_time_ns)
    return res.results


def kernel(x1, x2, x3, W1, g1, b1, W2, g2, b2, W3, g3, b3, trace=False):
    l1nc, passnc = _get_kernels()
    LAST_EXEC_NS.clear()

    f32 = np.float32
    xs = [np.ascontiguousarray(np.asarray(x, f32).reshape(2, INCH, HW))
          for x in (x1, x2, x3)]
    Ws = [np.ascontiguousarray(np.asarray(W, f32).T) for W in (W1, W2, W3)]
    gs = [np.asarray(g, f32) for g in (g1, g2, g3)]
    bs = [np.asarray(b, f32) for b in (b1, b2, b3)]

    def prm_cols(qi, ki, v1i, v2i):
        # columns: gq, bq, gk, bk, gv1, bv1, gv2, bv2
        return np.ascontiguousarray(np.stack(
            [gs[qi], bs[qi], gs[ki], bs[ki],
             gs[v1i], bs[v1i], gs[v2i], bs[v2i]], axis=1))

    # ---- Launch 1: A12 (Q1,K2) s0h0,s0h1,s1h0,s1h1; A13 (Q1,K3) same ----
    # extra slots (xv1, xv2) per core:
    #   c0: Q2 s0, V1 s0   c1: Q3 s0, V2 s0   c2: Q2 s1, V1 s1   c3: Q3 s1, V2 s1
    #   c4: K1 s0, V3 s0   c5: K1 s1, V3 s1   c6,c7: duplicates (unused)
    # (Q* use W1/g1/b1; K* use W2; V* use W3)
    extras = [
        ((xs[1], 0, 0), (xs[0], 2, 0)),   # (x, W-index, sample) pairs
        ((xs[2], 0, 0), (xs[1], 2, 0)),
        ((xs[1], 0, 1), (xs[0], 2, 1)),
        ((xs[2], 0, 1), (xs[1], 2, 1)),
        ((xs[0], 1, 0), (xs[2], 2, 0)),
        ((xs[0], 1, 1), (xs[2], 2, 1)),
        ((xs[0], 1, 0), (xs[2], 2, 0)),
        ((xs[0], 1, 1), (xs[2], 2, 1)),
    ]
    in_maps = []
    for core in range(8):
        mat = 0 if core < 4 else 1        # 0: A12 (K=K2), 1: A13 (K=K3)
        s = (core // 2) % 2
        h = core % 2
        xk_arr = xs[1][s] if mat == 0 else xs[2][s]
        (xe1, wi1, se1), (xe2, wi2, se2) = extras[core]
        in_maps.append({
            "xq": xs[0][s],
            "xqr": np.ascontiguousarray(xs[0][s][:, h * L1_R:(h + 1) * L1_R]),
            "xk": xk_arr,
            "xv1": xe1[se1],
            "xv2": xe2[se2],
            "wqT": Ws[0], "wkT": Ws[1] if mat == 0 else Ws[1],
            "wv1T": Ws[wi1], "wv2T": Ws[wi2],
            "prm": np.ascontiguousarray(np.stack(
                [gs[0], bs[0], gs[1], bs[1],
                 gs[wi1], bs[wi1], gs[wi2], bs[wi2]], axis=1)),
        })
    r1 = _run(l1nc, in_maps, trace)

    # Collect branch products [sample][name] -> [32, 4096]
    K2 = [r1[0]["kfull"], r1[2]["kfull"]]
    K3 = [r1[4]["kfull"], r1[6]["kfull"]]
    Q2 = [r1[0]["vout1"], r1[2]["vout1"]]
    Q3 = [r1[1]["vout1"], r1[3]["vout1"]]
    K1 = [r1[4]["vout1"], r1[5]["vout1"]]
    V1 = [r1[0]["vout2"], r1[2]["vout2"]]
    V2 = [r1[1]["vout2"], r1[3]["vout2"]]
    V3 = [r1[4]["vout2"], r1[5]["vout2"]]

    def partials(res, cores, vec):
        return np.sum([res[c]["out"][:, :, vec].reshape(HW) for c in cores], axis=0)

    u12 = [partials(r1, (0, 1), 0), partials(r1, (2, 3), 0)]
    p3b = [partials(r1, (4, 5), 0), partials(r1, (6, 7), 0)]

    ones = np.ones(LN_R, f32)

    def pass_launch(Q, K, v0s, v1s):
        """Q, K, v0s, v1s: per-sample arrays; returns (res0, res1) summed."""
        ims = []
        for core in range(8):
            s, quarter = core // 4, core % 4
            r0 = quarter * LN_R
            ims.append({
                "q": np.ascontiguousarray(Q[s][:, r0:r0 + LN_R]),
                "k": np.ascontiguousarray(K[s]),
                "v": np.ascontiguousarray(
                    np.stack([v0s[s][r0:r0 + LN_R], v1s[s][r0:r0 + LN_R]], axis=1)),
            })
        r = _run(passnc, ims, trace)
        o0 = [partials(r, range(0, 4), 0), partials(r, range(4, 8), 0)]
        o1 = [partials(r, range(0, 4), 1), partials(r, range(4, 8), 1)]
        return o0, o1

    ones2 = [ones * 0 + 1, ones * 0 + 1]
    onesHW = [np.ones(HW, f32), np.ones(HW, f32)]

    # L2: A23 = sm(Q2, K3); colsum -> colsum23, step(u12) -> w23 (= p3a)
    colsum23, w23 = pass_launch(Q2, K3, onesHW, u12)
    # L3: A32 = sm(Q3, K2); step(w23) -> w32, step(colsum23) -> p2
    w32, p2 = pass_launch(Q3, K2, w23, colsum23)
    # L4: A21 = sm(Q2, K1); step(w32) -> p1a, step(u12) -> p1b
    p1a, p1b = pass_launch(Q2, K1, w32, u12)

    def bc(vecs):
        v = np.stack(vecs).astype(f32)  # [2, HW]
        return np.broadcast_to(v[:, None, :], (2, C, HW)).reshape(2, C, 64, 64).copy()

    def vv(Vs):
        return np.stack(Vs).astype(f32).reshape(2, C, 64, 64)

    return (bc(p1a), bc(p1b), bc(p2), bc(w23), bc(p3b), vv(V1), vv(V2), vv(V3))

